# revision 12
# baseline (speedup 1.0000x reference)
"""Multi-head attention (B=2, H=16, S=4096, D=64, fp16) on 8 TRN2 NeuronCores.

Sharding: the 32 (b, h) head-slices are split 4-per-core (data/head
parallel, no cross-core communication). Each core runs a flash-attention
style kernel over its 4 heads.

Per-head algorithm (transposed-scores layout, no on-device transposes in
the hot loop):
  - Host pre-lays-out inputs: QT[d, s] = Q^T, KTp[d, j*128+p] = K[p*32+j, d]
    (a t-permutation that makes the V load contiguous), and VA = [V | 1]
    (ones column => the PV matmul also accumulates the softmax normalizer).
    QT/KT are loaded twice (partitions 0-63 and 64-127) so score matmuls can
    be row-packed onto both halves of the PE array (concurrent execution,
    weight loads pull ahead).
  - scores^T tile [t=128, s=512] = KTp_tile.T @ QT_tile   (PE, K=64)
  - P^T = exp(scale * scores^T)  fp32->fp16                (ACT, reads PSUM,
    1536-wide activations over 3 PSUM banks)
  - out^T [65, s] += VA_tile.T @ P^T_tile                  (PE, K=128; each
    VA stationary is loaded once and reused for the two 512-chunks of an
    s-window — the second matmul is marked non-self-loading)
    row 64 of out^T = sum_t P^T[t, s] = softmax denominator.
  - fixup per 1024-wide s-window: copy PSUM->SBUF, PE-transpose to
    [s=128, 65] blocks, reciprocal of col 64, per-partition scalar multiply,
    DMA out [s, d].

The emission runs a one-window software pipeline: while window w's scores
stream through PE->ACT/DVE, the PV matmuls consume window w-1's probs
(already in SBUF). That keeps the exp engines continuously fed and makes
both matmuls of each weight-sharing PV pair schedulable back-to-back.
`verify_ldw_pairs` checks the final PE order for every non-self-loading
matmul at build time.

exp is split across TWO engines: ACT (activation Exp, the only native exp,
1 elem/cycle/lane) takes most slices; DVE_PER_WIN slices per window run on
the otherwise-idle Vector engine as a magic-constant fast exp (see the
A_EXP/B_EXP comment) whose ~1.7% rms sawtooth error on the offloaded
fraction keeps total output error ~1e-2, inside the 2e-2 budget.

G=2 (not 3) so the row-packed score matmuls always issue as clean
concurrent pairs (an odd group size leaves half the PE array idle for the
third matmul).

Softmax skips max-subtraction: scores ~ N(0,1) after scaling, so fp32
exp/sum are numerically safe (|score*scale| < ~7 << 88).
"""

from contextlib import ExitStack

import numpy as np

import concourse.bass as bass
import concourse.tile as tile
from concourse import bacc, mybir
from concourse.bass_utils import run_bass_kernel_spmd
from concourse.masks import make_identity

B, H, S, D = 2, 16, 4096, 64
N_CORES = 8
HPC = (B * H) // N_CORES  # heads per core
SCALE = float(D) ** -0.5
SQ = 512  # s-chunk width (one PSUM bank of fp32)
G = 2  # t-tiles (PSUM banks) per exp group (even => score pairs pack cleanly)
WIN = 2 * SQ  # s-window: two chunks share each loaded PV stationary

ROWPACK_SCORES = True  # tile_position row-packed scores matmuls
PV_LDW_DEDUP = True  # share one weight load across each PV chunk pair
WARMUP = True  # HAM warmup matmul block

# --- DVE-offloaded exp (Schraudolph / magic-constant fast exp) ---
# ACT is the exp bottleneck (1 elem/cycle/lane @1.2GHz, no accel modes), so a
# fraction of exp slices runs on the otherwise-idle DVE instead:
#   v = score * A_EXP + B_EXP   computed in fp32 (one tensor_scalar op)
# With B_EXP offset by 2^23, v lands in [2^23, 2^24) where the fp32 mantissa
# is integer-valued, so the fp32 ADD itself performs round-to-nearest and the
# LOW 16 BITS of v are exactly the fp16 bit pattern (1+frac)*2^(t-C), the
# linear-interp approximation of exp(score*SCALE). The PV matmul reads those
# through a stride-2 fp16 view. The sawtooth rel-error is centered (MU term,
# zero geometric mean so it cancels in the softmax on average): ~1.7% rms on
# the offloaded fraction -> sqrt(lambda)*1.7% output rel err.
_LOG2E = 1.4426950408889634
_MU = 2.0 - 1.0 / float(np.log(2.0)) - 0.5  # mean of log2(1+g)-g, g~U[0,1]
A_EXP = float(_LOG2E * SCALE * 1024.0)
B_EXP = float((15.0 - _MU) * 1024.0 + 2.0**23)
import os as _os

DVE_PER_WIN = int(
    _os.environ.get("DVE_PER_WIN", "6")
)  # of the 32 exp slices per window, how many go to DVE
_NSL = 2 * (32 // G)  # exp slices per window (chunks x groups)
DVE_SLOTS = frozenset(int((k + 0.5) * _NSL / DVE_PER_WIN) for k in range(DVE_PER_WIN))


def attention_body(tc, qt, kt, va, o, heads, s, d):
    """Emit the per-core attention program.

    qt: [heads, d, s] fp16   Q^T per head
    kt: [heads, d, s] fp16   K^T per head, t-permuted (col j*128+p = row p*(s//128)+j)
    va: [heads, s, d+1] fp16 V with ones column
    o:  [heads, s, d] fp16   output
    """
    nc = tc.nc
    f32 = mybir.dt.float32
    f16 = mybir.dt.float16
    nt = s // 128  # number of 128-row t tiles
    nwin = s // WIN  # s windows per head
    nq = WIN // 128  # output row blocks per window

    groups = []
    t0 = 0
    while t0 < nt:
        gs = min(G, nt - t0)
        groups.append((t0, gs))
        t0 += gs

    with ExitStack() as ctx:
        qk_pool = ctx.enter_context(tc.tile_pool(name="qk", bufs=2))
        v_pool = ctx.enter_context(tc.tile_pool(name="v", bufs=2))
        # probs live from their exp (window w) until consumed by PV during
        # window w+1: ~1 window of slices in flight plus slack.
        n_dve = len([i for i in range(2 * len(groups)) if i in DVE_SLOTS])
        p_pool = ctx.enter_context(
            tc.tile_pool(name="p", bufs=(2 * len(groups) - n_dve) + 4)
        )
        # DVE-exp tiles hold fp16 pairs (prob in the low half of each fp32
        # word) so they are 2x wider.
        pf_pool = ctx.enter_context(tc.tile_pool(name="pf", bufs=n_dve + 4))
        # PSUM budget (8 banks of 512 fp32): scores 2 bufs x G banks = 4,
        # po 4 bufs x 1 bank = 4 (per-window: 4 PV half-accumulators, then the
        # two transpose scratch tiles reuse slots freed by the half-merge).
        ps_pool = ctx.enter_context(tc.tile_pool(name="ps", bufs=2, space="PSUM"))
        po_pool = ctx.enter_context(tc.tile_pool(name="po", bufs=4, space="PSUM"))
        fix_pool = ctx.enter_context(tc.tile_pool(name="fix", bufs=2))
        const_pool = ctx.enter_context(tc.tile_pool(name="const", bufs=1))

        ident = const_pool.tile([d + 1, d + 1], f32)
        make_identity(nc, ident)

        if WARMUP:
            # ~16 back-to-back matmuls trip the HAM activity window early so
            # the PE runs at 2.4 GHz instead of staying clock-gated at 1.2.
            warm_src = const_pool.tile([d + 1, SQ], f16)
            nc.vector.memset(warm_src, 1.0)
            warm_w = const_pool.tile([d + 1, d + 1], f16)
            nc.vector.memset(warm_w, 1.0)
            warm_ps = ps_pool.tile([128, G, SQ], f32, tag="ps")
            for i in range(16):
                nc.tensor.matmul(
                    warm_ps[: d + 1, 0, :],
                    lhsT=warm_w,
                    rhs=warm_src,
                    start=True,
                    stop=True,
                )

        # Per-head SBUF tiles, fetched lazily at head boundaries.
        head_tiles = {}

        def load_head(h):
            # Chunked loads ordered by first use so the first window's scores
            # only wait on the leading slices (Tile tracks byte-range deps).
            nck = 4
            cs = s // nck
            qt_sb = qk_pool.tile([128 if ROWPACK_SCORES else 64, s], f16, tag="qt")
            kt_sb = qk_pool.tile([128 if ROWPACK_SCORES else 64, s], f16, tag="kt")
            va_sb = v_pool.tile([128, nt, d + 1], f16, tag="va")
            va_src = va[h].rearrange("(p i) e -> p i e", p=128)
            rows = [0, 64] if ROWPACK_SCORES else [0]
            ick = nt // nck

            def kt_chunk(k):
                sl = slice(k * cs, (k + 1) * cs)
                for rp in rows:
                    nc.sync.dma_start(out=kt_sb[rp : rp + 64, sl], in_=kt[h][:, sl])

            def qt_chunk(k):
                sl = slice(k * cs, (k + 1) * cs)
                for rp in rows:
                    nc.sync.dma_start(out=qt_sb[rp : rp + 64, sl], in_=qt[h][:, sl])

            # kt chunk 0 + qt chunk 0 unblock the first window's scores; va is
            # first needed a window later; qt tails are needed last.
            kt_chunk(0)
            qt_chunk(0)
            for k in range(1, nck):
                kt_chunk(k)
            for k in range(nck):
                nc.sync.dma_start(
                    out=va_sb[:, k * ick : (k + 1) * ick, :],
                    in_=va_src[:, k * ick : (k + 1) * ick, :],
                )
            for k in range(1, nck):
                qt_chunk(k)
            head_tiles[h] = (qt_sb, kt_sb, va_sb)

        def emit_scores(h, w):
            """Scores + exp for window w of head h; returns per-group rhs
            descriptors: ('act', pt) or ('dve', pf) per chunk."""
            qt_sb, kt_sb, _ = head_tiles[h]
            w0 = w * WIN
            win_pts = []
            for gi, (t0, gs) in enumerate(groups):
                pts = []
                for c in (0, 1):
                    ps = ps_pool.tile([128, G, SQ], f32, tag="ps")
                    for g in range(gs):
                        t = t0 + g
                        rp = 64 * (t % 2) if ROWPACK_SCORES else 0
                        nc.tensor.matmul(
                            ps[:, g, :],
                            lhsT=kt_sb[rp : rp + 64, t * 128 : (t + 1) * 128],
                            rhs=qt_sb[
                                rp : rp + 64, w0 + c * SQ : w0 + (c + 1) * SQ
                            ],
                            start=True,
                            stop=True,
                            tile_position=(rp, 0) if ROWPACK_SCORES else None,
                        )
                    if (gi * 2 + c) in DVE_SLOTS:
                        pf = pf_pool.tile([128, G, 2 * SQ], f16, tag="pf")
                        nc.vector.tensor_scalar(
                            pf[:, :gs, :].bitcast(f32),
                            ps[:, :gs, :],
                            A_EXP,
                            B_EXP,
                            mybir.AluOpType.mult,
                            mybir.AluOpType.add,
                        )
                        pts.append(("dve", pf))
                    else:
                        pt = p_pool.tile([128, G, SQ], f16, tag="pt")
                        nc.scalar.activation(
                            pt[:, :gs, :],
                            ps[:, :gs, :],
                            mybir.ActivationFunctionType.Exp,
                            scale=SCALE,
                        )
                        pts.append(("act", pt))
                win_pts.append(pts)
            return win_pts

        def emit_pv_fixup(h, w, win_pts):
            """PV accumulation + normalize/store for window w of head h.

            The PV matmuls are row-packed like the scores: each (t, chunk)
            runs as two concurrent K=64 half-matmuls (tile_position rows 0 /
            64) into separate accumulators, so the PE array never switches
            between row-packed and full-K mode and weight loads always hide
            behind the other half. The halves merge for free: the fixup's
            PSUM->SBUF copy becomes a tensor_tensor add."""
            _, _, va_sb = head_tiles[h]
            w0 = w * WIN
            nqc = SQ // 128  # output row blocks per chunk
            pos = [
                [
                    po_pool.tile([d + 1, SQ], f32, tag="po", name=f"po{c}{hh}_{h}_{w}")
                    for hh in (0, 1)
                ]
                for c in (0, 1)
            ]
            for (t0, gs), pts in zip(groups, win_pts):
                for g in range(gs):
                    t = t0 + g
                    first = t == 0
                    last = t == nt - 1
                    for c in (0, 1):
                        kind, tl = pts[c]
                        for hh in (0, 1):
                            if kind == "dve":
                                rhs = tl[64 * hh : 64 * hh + 64, g, :].rearrange(
                                    "p (s two) -> p s two", two=2
                                )[:, :, 0]
                            else:
                                rhs = tl[64 * hh : 64 * hh + 64, g, :]
                            nc.tensor.matmul(
                                pos[c][hh],
                                lhsT=va_sb[64 * hh : 64 * hh + 64, t, :],
                                rhs=rhs,
                                start=first,
                                stop=last,
                                tile_position=(64 * hh, 0),
                            )

            # Per-chunk fixup chains so each PSUM bank frees as early as
            # possible (the po pool slot gates the next window's PV).
            o16 = fix_pool.tile([128, nq, d], f16, tag="o16")
            for c in (0, 1):
                # DVE has a single PSUM read port, so the half-merge is a
                # copy (frees bank h0) then an add with one PSUM operand.
                osb0 = fix_pool.tile([d + 1, SQ], f32, tag=f"osbA{c}")
                nc.vector.tensor_copy(osb0, pos[c][0])
                osb = fix_pool.tile([d + 1, SQ], f32, tag=f"osb{c}")
                nc.vector.tensor_tensor(
                    out=osb, in0=osb0, in1=pos[c][1], op=mybir.AluOpType.add
                )
                pt4 = po_pool.tile([128, nqc, 128], f32, tag="po")
                for qq in range(nqc):
                    nc.tensor.transpose(
                        pt4[:, qq, 0 : d + 1],
                        osb[:, qq * 128 : (qq + 1) * 128],
                        ident,
                    )
                rec = fix_pool.tile([128, nqc], f32, tag=f"rec{c}")
                nc.vector.reciprocal(rec, pt4[:, :, d])
                nc.vector.tensor_tensor(
                    out=o16[:, c * nqc : (c + 1) * nqc, :],
                    in0=pt4[:, :, 0:d],
                    in1=rec.unsqueeze(2).broadcast_to([128, nqc, d]),
                    op=mybir.AluOpType.mult,
                )
            nc.sync.dma_start(
                out=o[h, w0 : w0 + WIN, :].rearrange("(q p) d -> p q d", p=128),
                in_=o16,
            )

        windows = [(h, w) for h in range(heads) for w in range(nwin)]
        prev = None  # (h, w, win_pts) pending PV
        for i, (h, w) in enumerate(windows):
            if w == 0:
                load_head(h)
            win_pts = emit_scores(h, w)
            if prev is not None:
                emit_pv_fixup(*prev)
            prev = (h, w, win_pts)
        emit_pv_fixup(*prev)


def strip_redundant_ldweights(nc, strip=True):
    """Tile legalization emits one InstLdweights before every (non-transpose)
    matmul. When a load targets weights already resident and carries no
    semaphore traffic, drop it. Residency is tracked PER ROW-GROUP: a
    tile_position'd (row-packed) load only replaces the weights in its own
    row half, so interleaved h0/h64 loads both stay resident. A full-width
    load or a transpose matmul (which self-loads its input) clobbers
    everything. The same walk verifies every matmul's stationary operand
    against the resident weights of its row half."""

    def row_key(ins):
        tp = getattr(ins, "tile_position", None)
        return tp[0] if tp is not None else None  # None => full array

    removed = 0
    for f in nc.m.functions:
        for bb in f.blocks:
            insts = list(bb.instructions)
            keep = []
            resident = {}  # row base (or 'F') -> weights string
            changed = False
            for ins in insts:
                if isinstance(ins, mybir.InstLdweights):
                    w = str(ins.ins[0])
                    row = row_key(ins)
                    key = "F" if row is None else row
                    if (
                        strip
                        and resident.get(key) == w
                        and not ins.has_wait()
                        and not ins.has_update()
                    ):
                        removed += 1
                        changed = True
                        continue
                    if key == "F":
                        resident = {"F": w}
                    else:
                        resident.pop("F", None)
                        resident[key] = w
                elif isinstance(ins, mybir.InstMatmult):
                    if ins.is_transpose:
                        resident = {}  # transpose loads its input into the array
                    else:
                        w = str(ins.ins[1])
                        row = row_key(ins)
                        key = "F" if row is None else row
                        assert resident.get(key) == w, (
                            f"{ins.name}: stationary mismatch (row {key})\n"
                            f"loaded: {resident.get(key)}\nneeds:  {w}"
                        )
                keep.append(ins)
            if changed:
                bb.instructions = keep
    return removed


def build_program(heads=HPC, s=S, d=D):
    nc = bacc.Bacc(
        "TRN2", target_bir_lowering=False, debug=False, num_devices=N_CORES
    )
    qt = nc.dram_tensor("qt", [heads, d, s], mybir.dt.float16, kind="ExternalInput").ap()
    kt = nc.dram_tensor("kt", [heads, d, s], mybir.dt.float16, kind="ExternalInput").ap()
    va = nc.dram_tensor(
        "va", [heads, s, d + 1], mybir.dt.float16, kind="ExternalInput"
    ).ap()
    o = nc.dram_tensor("o", [heads, s, d], mybir.dt.float16, kind="ExternalOutput").ap()
    with tile.TileContext(nc) as tc:
        attention_body(tc, qt, kt, va, o, heads, s, d)
    if PV_LDW_DEDUP:
        strip_redundant_ldweights(nc)
    nc.compile()
    strip_redundant_ldweights(nc, strip=False)  # verify only
    return nc


def prep_core_inputs(Qc, Kc, Vc):
    """Host-side layout prep for one core's [heads, s, d] fp16 slices."""
    heads, s, d = Qc.shape
    qt = np.ascontiguousarray(Qc.transpose(0, 2, 1))
    k4 = Kc.reshape(heads, 128, s // 128, d)
    kt = np.ascontiguousarray(k4.transpose(0, 3, 2, 1)).reshape(heads, d, s)
    va = np.concatenate([Vc, np.ones((heads, s, 1), np.float16)], axis=2)
    return {"qt": qt, "kt": kt, "va": np.ascontiguousarray(va)}


_cache = {}


def kernel(Q, K, V):
    Q = np.asarray(Q, dtype=np.float16)
    K = np.asarray(K, dtype=np.float16)
    V = np.asarray(V, dtype=np.float16)
    b, h, s, d = Q.shape
    assert (b, h, s, d) == (B, H, S, D)

    if "nc" not in _cache:
        _cache["nc"] = build_program()
    nc = _cache["nc"]

    Qf = Q.reshape(b * h, s, d)
    Kf = K.reshape(b * h, s, d)
    Vf = V.reshape(b * h, s, d)
    in_maps = [
        prep_core_inputs(
            Qf[c * HPC : (c + 1) * HPC],
            Kf[c * HPC : (c + 1) * HPC],
            Vf[c * HPC : (c + 1) * HPC],
        )
        for c in range(N_CORES)
    ]
    res = run_bass_kernel_spmd(nc, in_maps, core_ids=list(range(N_CORES)))
    outs = [res.results[c]["o"] for c in range(N_CORES)]
    return np.concatenate(outs, axis=0).reshape(b, h, s, d)



# revision 13
# speedup vs baseline: 1.1347x; 1.1347x over previous
"""Multi-head attention (B=2, H=16, S=4096, D=64, fp16) on 8 TRN2 NeuronCores.

Sharding: the 32 (b, h) head-slices are split 4-per-core (data/head
parallel, no cross-core communication). Each core runs a flash-attention
style kernel over its 4 heads.

Per-head algorithm (transposed-scores layout, no on-device transposes in
the hot loop):
  - Host pre-lays-out inputs: QT[d, s] = Q^T, KTp[d, j*128+p] = K[p*32+j, d]
    (a t-permutation that makes the V load contiguous), and VA = [V | 1]
    (ones column => the PV matmul also accumulates the softmax normalizer).
    QT/KT are loaded twice (partitions 0-63 and 64-127) so score matmuls can
    be row-packed onto both halves of the PE array (concurrent execution,
    weight loads pull ahead).
  - scores^T tile [t=128, s=512] = KTp_tile.T @ QT_tile   (PE, K=64)
  - P^T = exp(scale * scores^T)  fp32->fp16                (ACT, reads PSUM,
    1536-wide activations over 3 PSUM banks)
  - out^T [65, s] += VA_tile.T @ P^T_tile                  (PE, K=128; each
    VA stationary is loaded once and reused for the two 512-chunks of an
    s-window — the second matmul is marked non-self-loading)
    row 64 of out^T = sum_t P^T[t, s] = softmax denominator.
  - fixup per 1024-wide s-window: copy PSUM->SBUF, PE-transpose to
    [s=128, 65] blocks, reciprocal of col 64, per-partition scalar multiply,
    DMA out [s, d].

The emission runs a one-window software pipeline: while window w's scores
stream through PE->ACT/DVE, the PV matmuls consume window w-1's probs
(already in SBUF). That keeps the exp engines continuously fed and makes
both matmuls of each weight-sharing PV pair schedulable back-to-back.
`verify_ldw_pairs` checks the final PE order for every non-self-loading
matmul at build time.

exp is split across TWO engines: ACT (activation Exp, the only native exp,
1 elem/cycle/lane) takes most slices; DVE_PER_WIN slices per window run on
the otherwise-idle Vector engine as a magic-constant fast exp (see the
A_EXP/B_EXP comment) whose ~1.7% rms sawtooth error on the offloaded
fraction keeps total output error ~1e-2, inside the 2e-2 budget.

G=2 (not 3) so the row-packed score matmuls always issue as clean
concurrent pairs (an odd group size leaves half the PE array idle for the
third matmul).

Softmax skips max-subtraction: scores ~ N(0,1) after scaling, so fp32
exp/sum are numerically safe (|score*scale| < ~7 << 88).
"""

from contextlib import ExitStack

import numpy as np

import concourse.bass as bass
import concourse.tile as tile
from concourse import bacc, mybir
from concourse.bass_utils import run_bass_kernel_spmd
from concourse.masks import make_identity

B, H, S, D = 2, 16, 4096, 64
N_CORES = 8
HPC = (B * H) // N_CORES  # heads per core
SCALE = float(D) ** -0.5
SQ = 512  # s-chunk width (one PSUM bank of fp32)
G = 2  # t-tiles (PSUM banks) per exp group (even => score pairs pack cleanly)
WIN = 2 * SQ  # s-window: two chunks share each loaded PV stationary

ROWPACK_SCORES = True  # tile_position row-packed scores matmuls
PV_LDW_DEDUP = True  # share one weight load across each PV chunk pair
WARMUP = True  # HAM warmup matmul block

# --- DVE-offloaded exp (Schraudolph / magic-constant fast exp) ---
# ACT is the exp bottleneck (1 elem/cycle/lane @1.2GHz, no accel modes), so a
# fraction of exp slices runs on the otherwise-idle DVE instead:
#   v = score * A_EXP + B_EXP   computed in fp32 (one tensor_scalar op)
# With B_EXP offset by 2^23, v lands in [2^23, 2^24) where the fp32 mantissa
# is integer-valued, so the fp32 ADD itself performs round-to-nearest and the
# LOW 16 BITS of v are exactly the fp16 bit pattern (1+frac)*2^(t-C), the
# linear-interp approximation of exp(score*SCALE). The PV matmul reads those
# through a stride-2 fp16 view. The sawtooth rel-error is centered (MU term,
# zero geometric mean so it cancels in the softmax on average): ~1.7% rms on
# the offloaded fraction -> sqrt(lambda)*1.7% output rel err.
_LOG2E = 1.4426950408889634
_MU = 2.0 - 1.0 / float(np.log(2.0)) - 0.5  # mean of log2(1+g)-g, g~U[0,1]
A_EXP = float(_LOG2E * SCALE * 1024.0)
B_EXP = float((15.0 - _MU) * 1024.0 + 2.0**23)
import os as _os

DVE_PER_WIN = int(
    _os.environ.get("DVE_PER_WIN", "6")
)  # of the 32 exp slices per window, how many go to DVE
_NSL = 2 * (32 // G)  # exp slices per window (chunks x groups)
DVE_SLOTS = frozenset(int((k + 0.5) * _NSL / DVE_PER_WIN) for k in range(DVE_PER_WIN))


def attention_body(tc, qt, kt, va, o, heads, s, d):
    """Emit the per-core attention program.

    qt: [heads, d, s] fp16   Q^T per head
    kt: [heads, d, s] fp16   K^T per head, t-permuted (col j*128+p = row p*(s//128)+j)
    va: [heads, s, d+1] fp16 V with ones column
    o:  [heads, s, d] fp16   output
    """
    nc = tc.nc
    f32 = mybir.dt.float32
    f16 = mybir.dt.float16
    nt = s // 128  # number of 128-row t tiles
    nwin = s // WIN  # s windows per head
    nq = WIN // 128  # output row blocks per window

    groups = []
    t0 = 0
    while t0 < nt:
        gs = min(G, nt - t0)
        groups.append((t0, gs))
        t0 += gs

    with ExitStack() as ctx:
        qk_pool = ctx.enter_context(tc.tile_pool(name="qk", bufs=2))
        v_pool = ctx.enter_context(tc.tile_pool(name="v", bufs=2))
        # probs live from their exp (window w) until consumed by PV during
        # window w+1: ~1 window of slices in flight plus slack.
        n_dve = len([i for i in range(2 * len(groups)) if i in DVE_SLOTS])
        p_pool = ctx.enter_context(
            tc.tile_pool(name="p", bufs=(2 * len(groups) - n_dve) + 4)
        )
        # DVE-exp tiles hold fp16 pairs (prob in the low half of each fp32
        # word) so they are 2x wider.
        pf_pool = ctx.enter_context(tc.tile_pool(name="pf", bufs=n_dve + 4))
        # PSUM budget (8 banks of 512 fp32): scores 2 bufs x G banks = 4,
        # po 4 bufs x 1 bank = 4 (per-window: 4 PV half-accumulators, then the
        # two transpose scratch tiles reuse slots freed by the half-merge).
        ps_pool = ctx.enter_context(tc.tile_pool(name="ps", bufs=2, space="PSUM"))
        po_pool = ctx.enter_context(tc.tile_pool(name="po", bufs=4, space="PSUM"))
        fix_pool = ctx.enter_context(tc.tile_pool(name="fix", bufs=2))
        const_pool = ctx.enter_context(tc.tile_pool(name="const", bufs=1))

        ident = const_pool.tile([d + 1, d + 1], f32)
        make_identity(nc, ident)

        if WARMUP:
            # ~16 back-to-back matmuls trip the HAM activity window early so
            # the PE runs at 2.4 GHz instead of staying clock-gated at 1.2.
            warm_src = const_pool.tile([d + 1, SQ], f16)
            nc.vector.memset(warm_src, 1.0)
            warm_w = const_pool.tile([d + 1, d + 1], f16)
            nc.vector.memset(warm_w, 1.0)
            warm_ps = ps_pool.tile([128, G, SQ], f32, tag="ps")
            for i in range(16):
                nc.tensor.matmul(
                    warm_ps[: d + 1, 0, :],
                    lhsT=warm_w,
                    rhs=warm_src,
                    start=True,
                    stop=True,
                )

        # Per-head SBUF tiles, fetched lazily at head boundaries.
        head_tiles = {}

        def load_head(h):
            # Chunked loads ordered by first use so the first window's scores
            # only wait on the leading slices (Tile tracks byte-range deps).
            nck = 4
            cs = s // nck
            qt_sb = qk_pool.tile([128 if ROWPACK_SCORES else 64, s], f16, tag="qt")
            kt_sb = qk_pool.tile([128 if ROWPACK_SCORES else 64, s], f16, tag="kt")
            va_sb = v_pool.tile([128, nt, d + 1], f16, tag="va")
            va_src = va[h].rearrange("(p i) e -> p i e", p=128)
            rows = [0, 64] if ROWPACK_SCORES else [0]
            ick = nt // nck

            def kt_chunk(k):
                sl = slice(k * cs, (k + 1) * cs)
                for rp in rows:
                    nc.sync.dma_start(out=kt_sb[rp : rp + 64, sl], in_=kt[h][:, sl])

            def qt_chunk(k):
                sl = slice(k * cs, (k + 1) * cs)
                for rp in rows:
                    nc.sync.dma_start(out=qt_sb[rp : rp + 64, sl], in_=qt[h][:, sl])

            # kt chunk 0 + qt chunk 0 unblock the first window's scores; va is
            # first needed a window later; qt tails are needed last.
            kt_chunk(0)
            qt_chunk(0)
            for k in range(1, nck):
                kt_chunk(k)
            for k in range(nck):
                nc.sync.dma_start(
                    out=va_sb[:, k * ick : (k + 1) * ick, :],
                    in_=va_src[:, k * ick : (k + 1) * ick, :],
                )
            for k in range(1, nck):
                qt_chunk(k)
            head_tiles[h] = (qt_sb, kt_sb, va_sb)

        def emit_score_slice(rec, gi, c):
            """Scores + exp for one (group, chunk) slice of rec's window."""
            qt_sb, kt_sb, _ = head_tiles[rec["h"]]
            w0 = rec["w"] * WIN
            t0, gs = groups[gi]
            ps = ps_pool.tile([128, G, SQ], f32, tag="ps")
            for g in range(gs):
                t = t0 + g
                rp = 64 * (t % 2) if ROWPACK_SCORES else 0
                nc.tensor.matmul(
                    ps[:, g, :],
                    lhsT=kt_sb[rp : rp + 64, t * 128 : (t + 1) * 128],
                    rhs=qt_sb[rp : rp + 64, w0 + c * SQ : w0 + (c + 1) * SQ],
                    start=True,
                    stop=True,
                    tile_position=(rp, 0) if ROWPACK_SCORES else None,
                )
            if (gi * 2 + c) in DVE_SLOTS:
                pf = pf_pool.tile([128, G, 2 * SQ], f16, tag="pf")
                nc.vector.tensor_scalar(
                    pf[:, :gs, :].bitcast(f32),
                    ps[:, :gs, :],
                    A_EXP,
                    B_EXP,
                    mybir.AluOpType.mult,
                    mybir.AluOpType.add,
                )
                return ("dve", pf)
            pt = p_pool.tile([128, G, SQ], f16, tag="pt")
            nc.scalar.activation(
                pt[:, :gs, :],
                ps[:, :gs, :],
                mybir.ActivationFunctionType.Exp,
                scale=SCALE,
            )
            return ("act", pt)

        def emit_pv_group(rec, gi, c):
            """PV quota for one (group, chunk) of rec's window.

            Row-packed like the scores: each (t, chunk) runs as two
            concurrent K=64 half-matmuls (tile_position rows 0/64) into
            separate accumulators, so the PE array never switches between
            row-packed and full-K mode and weight loads always hide behind
            the other half."""
            _, _, va_sb = head_tiles[rec["h"]]
            if rec["pos"] is None:
                rec["pos"] = [
                    [
                        po_pool.tile(
                            [d + 1, SQ],
                            f32,
                            tag="po",
                            name=f"po{cc}{hh}_{rec['h']}_{rec['w']}",
                        )
                        for hh in (0, 1)
                    ]
                    for cc in (0, 1)
                ]
            t0, gs = groups[gi]
            kind, tl = rec["pts"][gi][c]
            for g in range(gs):
                t = t0 + g
                first = t == 0
                last = t == nt - 1
                for hh in (0, 1):
                    if kind == "dve":
                        rhs = tl[64 * hh : 64 * hh + 64, g, :].rearrange(
                            "p (s two) -> p s two", two=2
                        )[:, :, 0]
                    else:
                        rhs = tl[64 * hh : 64 * hh + 64, g, :]
                    nc.tensor.matmul(
                        rec["pos"][c][hh],
                        lhsT=va_sb[64 * hh : 64 * hh + 64, t, :],
                        rhs=rhs,
                        start=first,
                        stop=last,
                        tile_position=(64 * hh, 0),
                    )

        def emit_fixup(rec):
            """Normalize/store for rec's window after its PV completes."""
            h, w, pos = rec["h"], rec["w"], rec["pos"]
            w0 = w * WIN
            nqc = SQ // 128  # output row blocks per chunk
            o16 = fix_pool.tile([128, nq, d], f16, tag="o16")
            for c in (0, 1):
                # DVE has a single PSUM read port, so the half-merge is a
                # copy (frees bank h0) then an add with one PSUM operand.
                osb0 = fix_pool.tile([d + 1, SQ], f32, tag=f"osbA{c}")
                nc.vector.tensor_copy(osb0, pos[c][0])
                osb = fix_pool.tile([d + 1, SQ], f32, tag=f"osb{c}")
                nc.vector.tensor_tensor(
                    out=osb, in0=osb0, in1=pos[c][1], op=mybir.AluOpType.add
                )
                pt4 = po_pool.tile([128, nqc, 128], f32, tag="po")
                for qq in range(nqc):
                    nc.tensor.transpose(
                        pt4[:, qq, 0 : d + 1],
                        osb[:, qq * 128 : (qq + 1) * 128],
                        ident,
                    )
                rec_t = fix_pool.tile([128, nqc], f32, tag=f"rec{c}")
                nc.vector.reciprocal(rec_t, pt4[:, :, d])
                nc.vector.tensor_tensor(
                    out=o16[:, c * nqc : (c + 1) * nqc, :],
                    in0=pt4[:, :, 0:d],
                    in1=rec_t.unsqueeze(2).broadcast_to([128, nqc, d]),
                    op=mybir.AluOpType.mult,
                )
            nc.sync.dma_start(
                out=o[h, w0 : w0 + WIN, :].rearrange("(q p) d -> p q d", p=128),
                in_=o16,
            )

        windows = [(h, w) for h in range(heads) for w in range(nwin)]
        prev = None  # pending-PV window record
        for h, w in windows:
            if w == 0:
                load_head(h)
            cur = {"h": h, "w": w, "pts": [], "pos": None}
            # Interleave per (group, chunk): the scores+exp of this window
            # with the previous window's PV quota for the same slice, so the
            # Tile scheduler's static PE order alternates 1 score pair with
            # 2 PV pairs per beat and never starves either stream.
            for gi in range(len(groups)):
                pts = []
                for c in (0, 1):
                    pts.append(emit_score_slice(cur, gi, c))
                    if prev is not None:
                        emit_pv_group(prev, gi, c)
                cur["pts"].append(pts)
            if prev is not None:
                emit_fixup(prev)
            prev = cur
        for gi in range(len(groups)):
            for c in (0, 1):
                emit_pv_group(prev, gi, c)
        emit_fixup(prev)


def strip_redundant_ldweights(nc, strip=True):
    """Tile legalization emits one InstLdweights before every (non-transpose)
    matmul. When a load targets weights already resident and carries no
    semaphore traffic, drop it. Residency is tracked PER ROW-GROUP: a
    tile_position'd (row-packed) load only replaces the weights in its own
    row half, so interleaved h0/h64 loads both stay resident. A full-width
    load or a transpose matmul (which self-loads its input) clobbers
    everything. The same walk verifies every matmul's stationary operand
    against the resident weights of its row half."""

    def row_key(ins):
        tp = getattr(ins, "tile_position", None)
        return tp[0] if tp is not None else None  # None => full array

    removed = 0
    for f in nc.m.functions:
        for bb in f.blocks:
            insts = list(bb.instructions)
            keep = []
            resident = {}  # row base (or 'F') -> weights string
            changed = False
            for ins in insts:
                if isinstance(ins, mybir.InstLdweights):
                    w = str(ins.ins[0])
                    row = row_key(ins)
                    key = "F" if row is None else row
                    if (
                        strip
                        and resident.get(key) == w
                        and not ins.has_wait()
                        and not ins.has_update()
                    ):
                        removed += 1
                        changed = True
                        continue
                    if key == "F":
                        resident = {"F": w}
                    else:
                        resident.pop("F", None)
                        resident[key] = w
                elif isinstance(ins, mybir.InstMatmult):
                    if ins.is_transpose:
                        resident = {}  # transpose loads its input into the array
                    else:
                        w = str(ins.ins[1])
                        row = row_key(ins)
                        key = "F" if row is None else row
                        assert resident.get(key) == w, (
                            f"{ins.name}: stationary mismatch (row {key})\n"
                            f"loaded: {resident.get(key)}\nneeds:  {w}"
                        )
                keep.append(ins)
            if changed:
                bb.instructions = keep
    return removed


def build_program(heads=HPC, s=S, d=D):
    nc = bacc.Bacc(
        "TRN2", target_bir_lowering=False, debug=False, num_devices=N_CORES
    )
    qt = nc.dram_tensor("qt", [heads, d, s], mybir.dt.float16, kind="ExternalInput").ap()
    kt = nc.dram_tensor("kt", [heads, d, s], mybir.dt.float16, kind="ExternalInput").ap()
    va = nc.dram_tensor(
        "va", [heads, s, d + 1], mybir.dt.float16, kind="ExternalInput"
    ).ap()
    o = nc.dram_tensor("o", [heads, s, d], mybir.dt.float16, kind="ExternalOutput").ap()
    with tile.TileContext(nc) as tc:
        attention_body(tc, qt, kt, va, o, heads, s, d)
    if PV_LDW_DEDUP:
        strip_redundant_ldweights(nc)
    nc.compile()
    strip_redundant_ldweights(nc, strip=False)  # verify only
    return nc


def prep_core_inputs(Qc, Kc, Vc):
    """Host-side layout prep for one core's [heads, s, d] fp16 slices."""
    heads, s, d = Qc.shape
    qt = np.ascontiguousarray(Qc.transpose(0, 2, 1))
    k4 = Kc.reshape(heads, 128, s // 128, d)
    kt = np.ascontiguousarray(k4.transpose(0, 3, 2, 1)).reshape(heads, d, s)
    va = np.concatenate([Vc, np.ones((heads, s, 1), np.float16)], axis=2)
    return {"qt": qt, "kt": kt, "va": np.ascontiguousarray(va)}


_cache = {}


def kernel(Q, K, V):
    Q = np.asarray(Q, dtype=np.float16)
    K = np.asarray(K, dtype=np.float16)
    V = np.asarray(V, dtype=np.float16)
    b, h, s, d = Q.shape
    assert (b, h, s, d) == (B, H, S, D)

    if "nc" not in _cache:
        _cache["nc"] = build_program()
    nc = _cache["nc"]

    Qf = Q.reshape(b * h, s, d)
    Kf = K.reshape(b * h, s, d)
    Vf = V.reshape(b * h, s, d)
    in_maps = [
        prep_core_inputs(
            Qf[c * HPC : (c + 1) * HPC],
            Kf[c * HPC : (c + 1) * HPC],
            Vf[c * HPC : (c + 1) * HPC],
        )
        for c in range(N_CORES)
    ]
    res = run_bass_kernel_spmd(nc, in_maps, core_ids=list(range(N_CORES)))
    outs = [res.results[c]["o"] for c in range(N_CORES)]
    return np.concatenate(outs, axis=0).reshape(b, h, s, d)

